# revision 36
# baseline (speedup 1.0000x reference)
"""Distributed Trainium2 kernel for nn_Aggregator (segment reduce + MLP + BN).

Strategy (8 NeuronCores, SPMD). Each segment is assigned to one core
(snake deal by size); only BatchNorm statistics cross cores (AllReduce of
2x128 floats). Two device streams per core:

  stream T (feat-major bf16)  [128, LT]: slots grouped into uniform-K
      regions (K = count rounded up to even, padding duplicates an edge,
      harmless for extrema); DVE computes per-slot min / max by log2
      fold with tensor_tensor at the 2x bf16 rate.
  stream E (edge-major fp8e4) [128, TE, 128] plus a host-built fp8
      one-hot [128, TE, 64]: tiles of 128 edges x 128 feats. PE computes
      per-window one-hot matmuls P^T @ [X | X^2] accumulating per-slot
      sum / sumsq (Act squares X on device). fp8 quantization of
      sum/sumsq features washes out through the 640-wide Linear.

  Per window: PSUM [64,256] -> bf16, scaled by 1/count (per-partition
  scalar at the 4x rate) -> mean | mean-square; PE transposes both to
  feat-major. Per 512-slot chunk: std epilogue, 4-block MLP matmul; the
  degree-embedding block is folded on the host into degw = (demb @ W_emb)
  [count] (a parameter-only transform); pad slots carry a correction
  column that cancels their std=sqrt(eps) contribution so BN sums stay
  exact. BN stats are AllReduced; normalize+ReLU via one activation op.

Host does layout only: permutation, padding, dtype casts, and
parameter-table preprocessing (demb @ W_emb, 1/count, degree clamp).
"""

import numpy as np
import ml_dtypes

import concourse.bass as bass
import concourse.bacc as bacc
import concourse.tile as tile
import concourse.mybir as mybir
from concourse import bass_utils

BF16 = ml_dtypes.bfloat16
FP8 = ml_dtypes.float8_e4m3
F32 = np.float32

NCORES = 8
D = 128
OUT = 128
WSLOT = 64         # slots per aggregation window (one PSUM group)
KGRAN = 2          # segment padding granularity in stream T (even folds)
CL_T = 4096        # stream-T chunk columns (bf16 elems per partition)
SCHUNK = 512       # slots per MLP/BN chunk
GW = SCHUNK // WSLOT
EPS_STD = 1e-5
EPS_BN = 1e-5

dt = mybir.dt


# ----------------------------------------------------------------------------
# Host-side planning (layout only)
# ----------------------------------------------------------------------------

class Plan:
    pass


def _round_up(x, m):
    return (x + m - 1) // m * m


def make_plan(index, N):
    E = index.shape[0]
    p = Plan()
    p.E, p.N = E, N

    counts = np.bincount(index, minlength=N)
    order = np.argsort(-counts, kind="stable")
    pos = np.arange(N)
    r, q = pos // NCORES, pos % NCORES
    snake = np.where(r % 2 == 0, q, NCORES - 1 - q)

    segs_c = [order[snake == c] for c in range(NCORES)]

    Kof = np.maximum(KGRAN, _round_up(np.maximum(counts, 1), KGRAN))

    allK = sorted(set(int(k) for k in np.unique(Kof)))
    S_K = {}
    for K in allK:
        m = max(int(np.sum(Kof[segs_c[c]] == K)) for c in range(NCORES))
        S_K[K] = m + (m % 2)  # even region sizes keep fold operands aligned
    S_total = sum(S_K.values())
    pad = (-S_total) % WSLOT
    K0 = allK[0]
    S_K[K0] += pad
    S_total += pad
    p.S_total = S_total
    p.allK = allK
    p.S_K = S_K

    p.slot_seg = np.full((NCORES, S_total), -1, np.int64)
    p.slot_cnt = np.zeros((NCORES, S_total), np.int64)
    slot_K = np.zeros(S_total, np.int64)
    off = 0
    for K in allK:
        nK = S_K[K]
        slot_K[off:off + nK] = K
        for c in range(NCORES):
            segs = segs_c[c][Kof[segs_c[c]] == K]
            p.slot_seg[c, off:off + len(segs)] = segs
            p.slot_cnt[c, off:off + len(segs)] = counts[segs]
        off += nK
    p.slot_K = slot_K
    p.LT = int(np.sum(slot_K))
    slot_tcol = np.zeros(S_total, np.int64)
    np.cumsum(slot_K[:-1], out=slot_tcol[1:])
    p.slot_tcol = slot_tcol

    # stream-T chunks of <= CL_T cols; regions (uniform-K runs) never split
    p.t_chunks = []
    cur_regions, cur_off, cur_cols = [], 0, 0
    s = 0
    while s < S_total:
        K = int(slot_K[s])
        cap = (CL_T - cur_cols) // K
        if cap == 0:
            p.t_chunks.append((cur_off, cur_cols, cur_regions))
            cur_off += cur_cols
            cur_regions, cur_cols = [], 0
            cap = CL_T // K
        run_end = s
        while run_end < S_total and slot_K[run_end] == K and run_end - s < cap:
            run_end += 1
        take = run_end - s
        cur_regions.append((cur_cols, take, K, s))
        cur_cols += take * K
        s = run_end
    if cur_regions:
        p.t_chunks.append((cur_off, cur_cols, cur_regions))
    p.regions_flat = []
    for (gcol, ncols, regions) in p.t_chunks:
        for (roff, rS, K, sbase) in regions:
            p.regions_flat.append((gcol + roff, rS, K, sbase))

    # windows
    p.nW = S_total // WSLOT
    ew = p.slot_cnt.reshape(NCORES, p.nW, WSLOT).sum(axis=2)
    p.T_w = np.maximum(1, -(-ew.max(axis=0) // 128)).astype(np.int64)
    p.TE = int(p.T_w.sum())
    p.tile_w0 = np.zeros(p.nW, np.int64)
    np.cumsum(p.T_w[:-1], out=p.tile_w0[1:])

    p.nSC = -(-S_total // SCHUNK)

    p.order_e = np.argsort(index, kind="stable")
    p.starts = np.zeros(N + 1, np.int64)
    np.cumsum(counts, out=p.starts[1:])
    return p


def make_core_arrays(p, c, x_bf, x_f8, index):
    """Per-core input arrays. x_bf: [E+1,128] bf16, x_f8: [E+1,128] fp8
    (row E is zeros)."""
    E = p.E
    order_e = p.order_e
    starts = p.starts
    S = p.S_total

    # ---- stream T: per region, transposed [K, S] so folds are dense ----
    eT = np.full(p.LT, E, np.int64)
    for (gcol, rS, K, sbase) in p.regions_flat:
        M = np.full((rS, K), E, np.int64)
        for j in range(rS):
            seg = p.slot_seg[c, sbase + j]
            if seg < 0:
                continue
            cnt = int(p.slot_cnt[c, sbase + j])
            ids = order_e[starts[seg]:starts[seg] + cnt]
            M[j, :cnt] = ids
            if cnt < K:
                M[j, cnt:] = ids[0] if cnt > 0 else E
        eT[gcol:gcol + rS * K] = M.T.ravel()
    xt = np.ascontiguousarray(x_bf[eT].T)  # [128, LT]

    # ---- stream E ----
    TE = p.TE
    eE = np.full(TE * 128, E, np.int64)
    sE = np.zeros(TE * 128, np.float32)
    for w in range(p.nW):
        o = int(p.tile_w0[w]) * 128
        for j in range(WSLOT):
            s = w * WSLOT + j
            seg = p.slot_seg[c, s]
            if seg < 0:
                continue
            cnt = int(p.slot_cnt[c, s])
            ids = order_e[starts[seg]:starts[seg] + cnt]
            eE[o:o + cnt] = ids
            sE[o:o + cnt] = j
            o += cnt
    xe8 = np.ascontiguousarray(
        x_f8[eE].reshape(TE, 128, 128).transpose(1, 0, 2))  # [128, TE, 128]
    oh = (sE.reshape(TE, 128)[:, :, None] ==
          np.arange(WSLOT, dtype=np.float32)[None, None, :])
    oh8 = np.ascontiguousarray(
        oh.astype(FP8).transpose(1, 0, 2))  # [128, TE, 64]

    # ---- per-slot rows ----
    cnt = p.slot_cnt[c]
    invc = (1.0 / np.maximum(cnt, 1)).astype(F32)
    invcw = invc.reshape(p.nW, WSLOT).T  # [64, nW]
    nW2 = (p.nW + 1) // 2
    invc2 = np.ones((128, nW2), F32)
    invc2[0:64, :] = invcw[:, 0::2]
    invc2[64:128, 0:p.nW // 2] = invcw[:, 1::2]
    return xt, xe8, oh8, np.ascontiguousarray(invc2)


def make_degw(p, c, deg_tab, W):
    """degw [128, S] bf16: per-slot deg-embedding contribution to h, with a
    correction on dead slots cancelling their std=sqrt(eps) term."""
    S = p.S_total
    W = np.asarray(W, np.float64)
    degw = np.zeros((S, 128), np.float64)
    segs = p.slot_seg[c]
    cnt = p.slot_cnt[c]
    live = segs >= 0
    deg = np.minimum(cnt[live], 99).astype(np.int64)
    degw[live] = deg_tab[deg]
    # dead slots: device computes std=sqrt(eps) (bf16) -> cancel W_std^T @ std
    std_pad = float(np.float32(np.sqrt(np.float32(EPS_STD))).astype(BF16))
    corr = -(std_pad * W[3 * 128:4 * 128, :].astype(np.float64).sum(axis=0))
    degw[~live] = corr
    return np.ascontiguousarray(degw.T.astype(BF16))  # [128, S]


# ----------------------------------------------------------------------------
# Device kernel builder
# ----------------------------------------------------------------------------

def build_kernel(p):
    nc = bacc.Bacc("TRN2", target_bir_lowering=False, debug=False,
                   num_devices=NCORES)
    S = p.S_total
    TE, nW = p.TE, p.nW
    TWmax = int(p.T_w.max())
    LT = p.LT

    xt_d = nc.dram_tensor("xt", [128, LT], dt.bfloat16, kind="ExternalInput")
    xe8_d = nc.dram_tensor("xe8", [128, TE, 128], dt.float8e4,
                           kind="ExternalInput")
    oh8_d = nc.dram_tensor("oh8", [128, TE, 64], dt.float8e4,
                           kind="ExternalInput")
    invc_d = nc.dram_tensor("invc", [128, (nW + 1) // 2], dt.float32,
                           kind="ExternalInput")
    degw_d = nc.dram_tensor("degw", [128, S], dt.bfloat16, kind="ExternalInput")
    w4_d = nc.dram_tensor("w4", [4, 128, 128], dt.bfloat16, kind="ExternalInput")
    gamma_d = nc.dram_tensor("gamma", [128, 1], dt.float32, kind="ExternalInput")
    beta_d = nc.dram_tensor("beta", [128, 1], dt.float32, kind="ExternalInput")
    ident64_d = nc.dram_tensor("ident64", [128, 128], dt.bfloat16,
                               kind="ExternalInput")
    hout_d = nc.dram_tensor("hout", [128, S], dt.bfloat16, kind="ExternalOutput")

    chunk_start_slot = [regions[0][3] for (_, _, regions) in p.t_chunks]

    with tile.TileContext(nc) as tc:
        import contextlib
        with contextlib.ExitStack() as ctx:
            cpool = ctx.enter_context(tc.tile_pool(name="const", bufs=1))
            tpool = ctx.enter_context(tc.tile_pool(name="tchunk", bufs=4))
            fpool = ctx.enter_context(tc.tile_pool(name="ftmp", bufs=1))
            epool = ctx.enter_context(tc.tile_pool(name="echunk", bufs=5))
            opool = ctx.enter_context(tc.tile_pool(name="onehot", bufs=4))
            wpool = ctx.enter_context(tc.tile_pool(name="wtmp", bufs=4))
            stpool = ctx.enter_context(tc.tile_pool(name="stats", bufs=1))
            hpool = ctx.enter_context(tc.tile_pool(name="hstage", bufs=2))
            mpool = ctx.enter_context(tc.tile_pool(name="misc", bufs=1))
            psw = ctx.enter_context(tc.tile_pool(name="psw", bufs=3, space="PSUM"))
            pst = ctx.enter_context(tc.tile_pool(name="pst", bufs=2, space="PSUM"))
            psh = ctx.enter_context(tc.tile_pool(name="psh", bufs=2, space="PSUM"))
            dram = ctx.enter_context(tc.tile_pool(name="dram", bufs=1, space="DRAM"))

            # --- constants ---
            ident64 = cpool.tile([128, 128], dt.bfloat16, tag="ident64")
            nc.sync.dma_start(ident64[:], ident64_d.ap())
            w4 = cpool.tile([128, 4 * 128], dt.bfloat16, tag="w4")
            nc.sync.dma_start(
                w4[:].rearrange("p (k f) -> p k f", k=4),
                w4_d.ap().rearrange("k p f -> p k f"))
            gamma = cpool.tile([128, 1], dt.float32, tag="gamma")
            nc.sync.dma_start(gamma[:], gamma_d.ap())
            beta = cpool.tile([128, 1], dt.float32, tag="beta")
            nc.sync.dma_start(beta[:], beta_d.ap())
            invc = cpool.tile([128, (nW + 1) // 2], dt.float32, tag="invc")
            nc.sync.dma_start(invc[:], invc_d.ap())
            degw = cpool.tile([128, S], dt.bfloat16, tag="degw")
            nc.sync.dma_start(degw[:], degw_d.ap())

            # --- persistent stats (feat-major) ---
            mnT = stpool.tile([128, S], dt.bfloat16, tag="mnT")
            mxT = stpool.tile([128, S], dt.bfloat16, tag="mxT")
            meanT = stpool.tile([128, S], dt.bfloat16, tag="meanT")
            msqT = stpool.tile([128, S], dt.bfloat16, tag="msqT")
            stdT = stpool.tile([128, S], dt.bfloat16, tag="stdT")
            hmS = stpool.tile([128, S], dt.float32, tag="hmS")
            sq_parts = stpool.tile([128, p.nSC], dt.float32, tag="sqp")
            hm_parts = stpool.tile([128, p.nSC], dt.float32, tag="hmp")

            # ---------------- stream T: min/max folds ----------------
            def emit_tchunk(gcol, ncols, regions):
                tch = tpool.tile([128, CL_T], dt.bfloat16, tag="tch")
                nc.gpsimd.dma_start(tch[:, :ncols],
                                    xt_d.ap()[:, gcol:gcol + ncols])
                for (roff, rS, K, sbase) in regions:
                    reg = tch[:, roff:roff + rS * K]
                    for op, dest, tg in ((mybir.AluOpType.min, mnT, "fmn"),
                                         (mybir.AluOpType.max, mxT, "fmx")):
                        tmp = fpool.tile([128, CL_T // 2], dt.bfloat16, tag=tg)
                        w = K
                        cur = reg
                        while w > 1:
                            half = (w + 1) // 2
                            nf = (w - half) * rS
                            i0 = cur[:, 0:nf]
                            i1 = cur[:, half * rS:w * rS]
                            if half == 1:
                                o = dest[:, sbase:sbase + rS]
                            else:
                                o = tmp[:, 0:nf]
                            nc.vector.tensor_tensor(out=o, in0=i0, in1=i1, op=op)
                            if half == 1:
                                break
                            if cur is reg:
                                cur = tmp
                            w = half

            # ---------------- stream E: windows ----------------
            wbufs = {}

            def emit_window_pre(w):
                t0 = int(p.tile_w0[w])
                Tw = int(p.T_w[w])
                xa = epool.tile([128, 2 * TWmax * 128], dt.float8e4, tag="xa")
                nc.sync.dma_start(
                    xa[:, 0:Tw * 128].rearrange("p (t c) -> p t c", c=128),
                    xe8_d.ap()[:, t0:t0 + Tw, :])
                # squares into the second half
                nc.scalar.activation(
                    out=xa[:, TWmax * 128:TWmax * 128 + Tw * 128],
                    in_=xa[:, 0:Tw * 128],
                    func=mybir.ActivationFunctionType.Square)
                # one-hot (host-built, fp8)
                oh = opool.tile([128, TWmax * 64], dt.float8e4, tag="oh")
                nc.sync.dma_start(
                    oh[:, 0:Tw * 64].rearrange("p (t f) -> p t f", f=64),
                    oh8_d.ap()[:, t0:t0 + Tw, :])
                wbufs[w] = (xa, oh)

            pair = {}

            def emit_window_mm(w):
                Tw = int(p.T_w[w])
                xa, oh = wbufs.pop(w)
                x3 = xa[:].rearrange("p (b t c) -> p b t c", b=2, c=128)
                h = w % 2
                if h == 0:
                    pair["ps"] = psw.tile([128, 256], dt.float32,
                                          name="pspair", tag="psw")
                ps = pair["ps"]
                ps3 = ps[h * 64:h * 64 + 64, :].rearrange(
                    "p (b c) -> p b c", b=2)
                for t in range(Tw):
                    nc.tensor.matmul(
                        out=ps3,
                        lhsT=oh[:, t * 64:(t + 1) * 64],
                        rhs=x3[:, :, t, :],
                        start=(t == 0), stop=(t == Tw - 1))
                if h == 0 and w != nW - 1:
                    return
                # pair complete (or final odd window): one wide fused
                # psum->sbuf copy + 1/count scale for both windows
                lo = w - h  # first window of the pair
                np_ = 64 * (h + 1)
                stw = wpool.tile([128, 256], dt.bfloat16, tag="stw")
                nc.scalar.activation(
                    out=stw[0:np_, :], in_=ps[0:np_, :],
                    func=mybir.ActivationFunctionType.Identity,
                    scale=invc[0:np_, w // 2:w // 2 + 1])
                pt = pst.tile([128, 128], dt.bfloat16, tag="pt")
                nc.tensor.transpose(out=pt[0:np_ if h else 128, 0:np_],
                                    in_=stw[0:np_, 0:128],
                                    identity=ident64[0:np_, 0:np_])
                pt2 = pst.tile([128, 128], dt.bfloat16, tag="pt")
                nc.tensor.transpose(out=pt2[0:np_ if h else 128, 0:np_],
                                    in_=stw[0:np_, 128:256],
                                    identity=ident64[0:np_, 0:np_])
                sl = slice(lo * 64, lo * 64 + np_)
                nc.scalar.copy(out=meanT[:, sl], in_=pt[0:128, 0:np_])
                nc.scalar.copy(out=msqT[:, sl], in_=pt2[0:128, 0:np_])

            # ---------------- per-SCHUNK epilogue + MLP ----------------
            def emit_mlp(ci):
                o0 = ci * SCHUNK
                cw = min(SCHUNK, S - o0)
                # std = sqrt(relu(msq - mean^2) + eps)
                v = hpool.tile([128, SCHUNK], dt.bfloat16, tag="v")
                nc.vector.tensor_tensor(
                    out=v[:, 0:cw], in0=meanT[:, o0:o0 + cw],
                    in1=meanT[:, o0:o0 + cw], op=mybir.AluOpType.mult)
                nc.vector.tensor_tensor(
                    out=v[:, 0:cw], in0=msqT[:, o0:o0 + cw], in1=v[:, 0:cw],
                    op=mybir.AluOpType.subtract)
                nc.vector.tensor_scalar(
                    out=v[:, 0:cw], in0=v[:, 0:cw],
                    scalar1=0.0, scalar2=EPS_STD,
                    op0=mybir.AluOpType.max, op1=mybir.AluOpType.add)
                nc.scalar.activation(
                    out=stdT[:, o0:o0 + cw], in_=v[:, 0:cw],
                    func=mybir.ActivationFunctionType.Sqrt)
                # MLP: h = sum_k W_k^T @ stat_k + degw
                ph = psh.tile([128, SCHUNK], dt.float32, tag="ph")
                stats = (meanT, mnT, mxT, stdT)
                for k in range(4):
                    nc.tensor.matmul(
                        out=ph[:, 0:cw],
                        lhsT=w4[:, k * 128:(k + 1) * 128],
                        rhs=stats[k][:, o0:o0 + cw],
                        start=(k == 0), stop=False)
                # degree-embedding term via identity matmul on the idle PE
                nc.tensor.matmul(
                    out=ph[:, 0:cw], lhsT=ident64[:],
                    rhs=degw[:, o0:o0 + cw], start=False, stop=True)
                nc.scalar.copy(out=hmS[:, o0:o0 + cw], in_=ph[:, 0:cw])
                hsq = hpool.tile([128, SCHUNK], dt.bfloat16, tag="hsq")
                nc.scalar.activation(
                    out=hsq[:, 0:cw], in_=hmS[:, o0:o0 + cw],
                    func=mybir.ActivationFunctionType.Square,
                    accum_out=sq_parts[:, ci:ci + 1])
                nc.scalar.activation(
                    out=hsq[:, 0:cw], in_=hmS[:, o0:o0 + cw],
                    func=mybir.ActivationFunctionType.Identity,
                    accum_out=hm_parts[:, ci:ci + 1])

            # ---- interleaved emission (BN AllReduce split in two) ----
            SPLITC = p.nSC * 2 // 3  # 0 when nSC < 2: skip early AR
            bounce_i1 = dram.tile([128, 2], dt.float32)
            bounce_o1 = dram.tile([128, 2], dt.float32)
            bounce_i2 = dram.tile([128, 2], dt.float32)
            bounce_o2 = dram.tile([128, 2], dt.float32)
            bn_a = mpool.tile([128, 2], dt.float32, tag="bna")
            if SPLITC == 0:
                nc.vector.memset(bn_a[:], 0.0)
                nc.gpsimd.dma_start(bounce_i1[:], bn_a[:])
                nc.gpsimd.collective_compute(
                    "AllReduce", mybir.AluOpType.add,
                    replica_groups=[list(range(NCORES))],
                    ins=[bounce_i1.opt()], outs=[bounce_o1.opt()])
            tix = 0
            LOOKAHEAD = 4
            for w in range(min(LOOKAHEAD, nW)):
                emit_window_pre(w)
            while tix < min(2, len(p.t_chunks)):
                emit_tchunk(*p.t_chunks[tix])
                tix += 1
            for ci in range(p.nSC):
                w0, w1 = ci * GW, min(ci * GW + GW, nW)
                for w in range(w0, w1):
                    if w + LOOKAHEAD < nW:
                        emit_window_pre(w + LOOKAHEAD)
                    emit_window_mm(w)
                    while (tix < len(p.t_chunks)
                           and chunk_start_slot[tix] < (w + 1) * WSLOT):
                        emit_tchunk(*p.t_chunks[tix])
                        tix += 1
                emit_mlp(ci)
                if SPLITC > 0 and ci == SPLITC - 1:
                    # early partial BN sums -> background AllReduce
                    nc.vector.tensor_reduce(
                        out=bn_a[:, 0:1], in_=hm_parts[:, 0:SPLITC],
                        axis=mybir.AxisListType.X, op=mybir.AluOpType.add)
                    nc.vector.tensor_reduce(
                        out=bn_a[:, 1:2], in_=sq_parts[:, 0:SPLITC],
                        axis=mybir.AxisListType.X, op=mybir.AluOpType.add)
                    nc.gpsimd.dma_start(bounce_i1[:], bn_a[:])
                    nc.gpsimd.collective_compute(
                        "AllReduce", mybir.AluOpType.add,
                        replica_groups=[list(range(NCORES))],
                        ins=[bounce_i1.opt()], outs=[bounce_o1.opt()])
            while tix < len(p.t_chunks):
                emit_tchunk(*p.t_chunks[tix])
                tix += 1

            # ---- rest of BN stats + final AllReduce ----
            bn_in = mpool.tile([128, 2], dt.float32, tag="bnin")
            nc.vector.tensor_reduce(
                out=bn_in[:, 0:1], in_=hm_parts[:, SPLITC:p.nSC],
                axis=mybir.AxisListType.X, op=mybir.AluOpType.add)
            nc.vector.tensor_reduce(
                out=bn_in[:, 1:2], in_=sq_parts[:, SPLITC:p.nSC],
                axis=mybir.AxisListType.X, op=mybir.AluOpType.add)
            nc.gpsimd.dma_start(bounce_i2[:], bn_in[:])
            nc.gpsimd.collective_compute(
                "AllReduce", mybir.AluOpType.add,
                replica_groups=[list(range(NCORES))],
                ins=[bounce_i2.opt()], outs=[bounce_o2.opt()])
            bn_o1 = mpool.tile([128, 2], dt.float32, tag="bno1")
            nc.gpsimd.dma_start(bn_o1[:], bounce_o1[:])
            bn_o2 = mpool.tile([128, 2], dt.float32, tag="bno2")
            nc.gpsimd.dma_start(bn_o2[:], bounce_o2[:])
            bn_out = mpool.tile([128, 2], dt.float32, tag="bnout")
            nc.vector.tensor_tensor(out=bn_out[:], in0=bn_o1[:], in1=bn_o2[:],
                                    op=mybir.AluOpType.add)

            inv_n = 1.0 / float(p.N)
            mu = mpool.tile([128, 1], dt.float32, tag="mu")
            nc.vector.tensor_scalar(out=mu[:], in0=bn_out[:, 0:1],
                                    scalar1=inv_n, scalar2=None,
                                    op0=mybir.AluOpType.mult)
            ex2 = mpool.tile([128, 1], dt.float32, tag="ex2")
            nc.vector.tensor_scalar(out=ex2[:], in0=bn_out[:, 1:2],
                                    scalar1=inv_n, scalar2=None,
                                    op0=mybir.AluOpType.mult)
            var = mpool.tile([128, 1], dt.float32, tag="var")
            nc.vector.tensor_tensor(out=var[:], in0=mu[:], in1=mu[:],
                                    op=mybir.AluOpType.mult)
            nc.vector.tensor_tensor(out=var[:], in0=ex2[:], in1=var[:],
                                    op=mybir.AluOpType.subtract)
            nc.vector.tensor_scalar(out=var[:], in0=var[:], scalar1=EPS_BN,
                                    scalar2=None, op0=mybir.AluOpType.add)
            sdv = mpool.tile([128, 1], dt.float32, tag="sdv")
            nc.scalar.activation(out=sdv[:], in_=var[:],
                                 func=mybir.ActivationFunctionType.Sqrt)
            istd = mpool.tile([128, 1], dt.float32, tag="istd")
            nc.vector.reciprocal(out=istd[:], in_=sdv[:])
            scl = mpool.tile([128, 1], dt.float32, tag="scl")
            nc.vector.tensor_tensor(out=scl[:], in0=gamma[:], in1=istd[:],
                                    op=mybir.AluOpType.mult)
            shf = mpool.tile([128, 1], dt.float32, tag="shf")
            nc.vector.tensor_tensor(out=shf[:], in0=mu[:], in1=scl[:],
                                    op=mybir.AluOpType.mult)
            nc.vector.tensor_tensor(out=shf[:], in0=beta[:], in1=shf[:],
                                    op=mybir.AluOpType.subtract)

            # ---- normalize + relu + out ----
            for ci in range(p.nSC):
                o0 = ci * SCHUNK
                cw = min(SCHUNK, S - o0)
                hs = hpool.tile([128, SCHUNK], dt.bfloat16, tag="hs")
                nc.scalar.activation(
                    out=hs[:, 0:cw], in_=hmS[:, o0:o0 + cw],
                    func=mybir.ActivationFunctionType.Relu,
                    scale=scl[:], bias=shf[:])
                nc.sync.dma_start(hout_d.ap()[:, o0:o0 + cw], hs[:, 0:cw])

    nc.compile()
    return nc


# ----------------------------------------------------------------------------
# Top-level
# ----------------------------------------------------------------------------

def prepare(inputs, index, deg_emb, W, gamma, beta, dim_size):
    N = int(dim_size)
    E = index.shape[0]
    index = np.asarray(index)
    p = make_plan(index, N)

    x = np.asarray(inputs)
    x_bf = np.empty((E + 1, 128), BF16)
    x_bf[:E] = x.astype(BF16)
    x_bf[E] = 0
    x_f8 = np.empty((E + 1, 128), FP8)
    x_f8[:E] = x.astype(FP8)
    x_f8[E] = 0

    Wf = np.asarray(W, F32)
    deg_tab = np.asarray(deg_emb, np.float64) @ Wf[4 * 128:5 * 128].astype(np.float64)

    TWmax = int(p.T_w.max())
    in_maps = []
    for c in range(NCORES):
        xt, xe8, oh8, invcw = make_core_arrays(p, c, x_bf, x_f8, index)
        m = {
            "xt": xt, "xe8": xe8, "oh8": oh8, "invc": invcw,
            "degw": make_degw(p, c, deg_tab, Wf),
            "w4": np.ascontiguousarray(
                Wf[:4 * 128].astype(BF16).reshape(4, 128, 128)),
            "gamma": np.asarray(gamma, F32).reshape(128, 1),
            "beta": np.asarray(beta, F32).reshape(128, 1),
            "ident64": np.eye(128, dtype=BF16),
        }
        in_maps.append(m)

    nc = build_kernel(p)
    prepare.last_plan = p

    def assemble(results):
        out = np.zeros((N, 128), F32)
        for c in range(NCORES):
            hT = np.asarray(results[c]["hout"], dtype=F32)  # [128, S]
            segs = p.slot_seg[c]
            mask = segs >= 0
            out[segs[mask]] = hT.T[mask]
        return out

    return nc, in_maps, assemble


def kernel(inputs, index, deg_emb, W, gamma, beta, dim_size):
    nc, in_maps, assemble = prepare(inputs, index, deg_emb, W, gamma, beta,
                                    dim_size)
    res = bass_utils.run_bass_kernel_spmd(
        nc, in_maps, core_ids=list(range(NCORES)))
    return assemble(res.results)
